# revision 12
# baseline (speedup 1.0000x reference)
"""Trainium2 Bass kernel: collaborative-filtering score (segment_reduce problem).

Math (per batch element b):
    ubf[u]    = masked mean over nonzero entries of rating_mtx[u, :]
    score[b]  = sum_u  S[user_b, u] * (R[u, item_b] - ubf[u])
    out[b]    = 5 * sigmoid(score[b] + user_bias[user_b] + item_bias[item_b] + gb)

Distribution: BATCH-sharded. Core k owns batch slice [k*1024, (k+1)*1024) and
gathers FULL 8192-wide fp16 rows of the (replicated) augmented similarity
table (by user idx) and of the transposed rating table (by item idx) with
NON-transposed dma_gather: each row lands whole on one partition, so a chunk
is [128 batch-rows x 8320 cols] and the score is a plain FREE-AXIS reduce:
    DVE/Pool:  ak += ubf_bc   (row-broadcast -(ubf-2.5), built once)
    DVE:       ak *= gk       (in-place products)
    ACT:       Copy+accum_out -> scores[:, k]  (the 8192-long dot, fp32 accum)
No PE matmuls, no score AllReduce: each core sigmoids its own scores and
writes its output shard (one transposed [8,128] contiguous DMA); the host
concatenates. 8x fewer gathered rows than user-sharding keeps the Pool
engine (~3us fixed per dma_gather) off the critical path.

ubf is computed u-sharded per the hint ("masked mean shards along n_users
rows with an all-gather of user_bias_fixed"): each core streams its fp8
natural-layout slice (R-2.5, exact in e4m3) split across the sync AND scalar
DMA rings (so the gather queues don't starve it), sums on ACT (Copy+accum),
counts nonzeros on DVE (mask+reduce) for most tiles and on ACT (Sign(x+2)
accum = 2cnt-I) for the rest, PE-transposes the [128,8] result to user
order, and a tiny fp16 AllGather distributes it. A dummy 256B AllGather is
issued first, AFTER the first `prefetch` chunk gather-pairs are queued on
Pool: the first collective absorbs the multi-core start skew while the DMA
queues prefetch ~17MB of gather data + the rnat stream, so the real
exchange costs only ~5us on an already-synced device.

Tables are fp16 (NOT bf16): S values are ~N(0, 0.011) so fp16's 10-bit
mantissa cuts quantization noise ~8x vs bf16; centered ratings and the
hi/lo-split biases are exact. Biases ride as 4 augmented fp16 columns
(8192..8195) folded into the same dot product.
"""

import sys
from dataclasses import dataclass

import numpy as np

if "/opt/trn_rl_repo" not in sys.path:
    sys.path.insert(0, "/opt/trn_rl_repo")


@dataclass(frozen=True)
class Cfg:
    n_users: int = 8192
    n_items: int = 4096
    batch: int = 8192
    n_cores: int = 8
    chunk: int = 128  # batch elems per gather chunk
    prefetch: int = 4  # gather chunk-pairs queued on Pool before the warm-up cc
    pool_adds: int = 4  # trailing chunks whose ubf-add runs on Pool, not DVE
    act_cnts: int = 2  # trailing rnat tiles whose count runs on ACT, not DVE

    @property
    def bl(self) -> int:  # batch elems per core
        return self.batch // self.n_cores

    @property
    def w(self) -> int:
        # gather-row width: n_users data cols + 4 bias cols, padded to a
        # multiple of 128 elems (fp16 -> 256B rows, dma_gather constraint)
        return ((self.n_users + 4 + 127) // 128) * 128


def build_program(cfg: Cfg):
    from concourse import bacc, mybir, tile
    from concourse.masks import make_identity

    f32 = mybir.dt.float32
    f16 = mybir.dt.float16
    f8 = mybir.dt.float8e4
    i16 = mybir.dt.int16
    Alu = mybir.AluOpType
    Act = mybir.ActivationFunctionType

    U, I, B, W = cfg.n_users, cfg.n_items, cfg.batch, cfg.w
    BL, CH = cfg.bl, cfg.chunk
    NCH = BL // CH  # main-loop chunks (8)
    ICN = CH // 16  # idx-table cols per chunk
    RT = BL // 128  # rnat row-tiles (8)
    IDXC = BL // 16
    groups = [list(range(cfg.n_cores))]

    nc = bacc.Bacc(
        None, target_bir_lowering=False, debug=False, num_swdge_queues=2
    )

    sim_t = nc.dram_tensor("sim_aug", [U, W], f16, kind="ExternalInput")
    rtt_t = nc.dram_tensor("ratt_aug", [I, W], f16, kind="ExternalInput")
    rnat_t = nc.dram_tensor("rnat", [BL, I], f8, kind="ExternalInput")
    uidx_t = nc.dram_tensor("uidx", [128, IDXC], i16, kind="ExternalInput")
    iidx_t = nc.dram_tensor("iidx", [128, IDXC], i16, kind="ExternalInput")
    out_t = nc.dram_tensor("out", [BL], f32, kind="ExternalOutput")

    with tile.TileContext(nc) as tc:
        with (
            tc.tile_pool(name="static", bufs=1) as st,
            tc.tile_pool(name="rstream", bufs=1) as rpool,
            tc.tile_pool(name="scr", bufs=1) as scr,
            tc.tile_pool(name="scr2", bufs=1) as scr2,
            tc.tile_pool(name="gpool", bufs=4) as gpool,
            tc.tile_pool(name="apool", bufs=3) as apool,
            tc.tile_pool(name="ps", bufs=1, space="PSUM") as psp,
            tc.tile_pool(name="dram", bufs=1, space="DRAM") as dram,
        ):
            # ---- static setup ----
            two_b = st.tile([128, 1], f32)
            nc.gpsimd.memset(two_b[:], 2.0)
            ident = st.tile([128, 128], f32)
            make_identity(nc, ident[:])
            uidx_sb = st.tile([128, IDXC], i16)
            nc.sync.dma_start(out=uidx_sb[:], in_=uidx_t[:])
            iidx_sb = st.tile([128, IDXC], i16)
            nc.sync.dma_start(out=iidx_sb[:], in_=iidx_t[:])

            # ---- rnat stream: all tiles up-front, alternating sync/scalar
            # DMA rings so the two gather queues can't starve it
            rts = []
            for j in range(RT):
                rt = rpool.tile([128, I], f8, name=f"rt{j}")
                eng = nc.sync if j % 2 == 0 else nc.scalar
                eng.dma_start(out=rt[:], in_=rnat_t[j * 128 : (j + 1) * 128, :])
                rts.append(rt)

            # ---- ubf local pass: sum on ACT; count on DVE (mask+reduce)
            # except the last `act_cnts` tiles, counted on ACT via
            # Sign(x+2.0): accum = 2*cnt - I (x is never 0).
            sum_acc = st.tile([128, RT], f32)
            cnt_acc = st.tile([128, RT], f32)
            dve_cnts = RT - cfg.act_cnts
            for j in range(RT):
                rt = rts[j]
                s1 = scr.tile([128, I], f8, name="s1")
                nc.scalar.activation(
                    out=s1[:], in_=rt[:], func=Act.Copy,
                    accum_out=sum_acc[:, j : j + 1],
                )
                if j < dve_cnts:
                    mk = scr2.tile([128, I], f16, name="mk")
                    nc.vector.tensor_scalar(
                        out=mk[:], in0=rt[:], scalar1=-2.5, scalar2=None,
                        op0=Alu.not_equal,
                    )
                    nc.vector.tensor_reduce(
                        out=cnt_acc[:, j : j + 1], in_=mk[:],
                        axis=mybir.AxisListType.X, op=Alu.add,
                    )
                else:
                    s2 = scr.tile([128, I], f8, name="s1")
                    nc.scalar.activation(
                        out=s2[:], in_=rt[:], func=Act.Sign, bias=two_b[:],
                        accum_out=cnt_acc[:, j : j + 1],
                    )
            if cfg.act_cnts:
                # convert the Sign-accumulated cols: cnt = (acc + I) / 2
                nc.vector.tensor_scalar(
                    out=cnt_acc[:, dve_cnts:], in0=cnt_acc[:, dve_cnts:],
                    scalar1=0.5, scalar2=float(I) / 2, op0=Alu.mult,
                    op1=Alu.add,
                )
            # ubf math (values are R-2.5; sum_R = sum_acc + 2.5*I):
            #   ubf  = sum_R / max(cnt, 1)      (0 when cnt==0)
            #   ubfn = -(ubf - 2.5) = 2.5 - ubf (cnt==0 -> sum_R==0 -> 2.5,
            #   matching the reference's adjusted = R - 0 in centered form)
            cntm = st.tile([128, RT], f32)
            nc.vector.tensor_scalar(
                out=cntm[:], in0=cnt_acc[:], scalar1=1.0, scalar2=None,
                op0=Alu.max,
            )
            nc.vector.reciprocal(out=cntm[:], in_=cntm[:])
            ubfn = st.tile([128, RT], f32)
            nc.vector.tensor_scalar(
                out=ubfn[:], in0=sum_acc[:], scalar1=2.5 * I, scalar2=None,
                op0=Alu.add,
            )
            nc.vector.tensor_tensor(
                out=ubfn[:], in0=ubfn[:], in1=cntm[:], op=Alu.mult
            )
            nc.vector.tensor_scalar(
                out=ubfn[:], in0=ubfn[:], scalar1=-1.0, scalar2=2.5,
                op0=Alu.mult, op1=Alu.add,
            )
            # transpose [128, RT] -> [RT, 128] so DRAM order is user order
            ps_t = psp.tile([RT, 128], f32, name="ps_t")
            nc.tensor.transpose(out=ps_t[:], in_=ubfn[:], identity=ident[:])
            ubfn_t = st.tile([RT, 128], f16)
            nc.vector.tensor_copy(out=ubfn_t[:], in_=ps_t[:])
            pd = dram.tile([1, BL], f16, name="ubf_part")
            nc.sync.dma_start(
                out=pd[:].rearrange("o (j p) -> (o j) p", j=RT), in_=ubfn_t[:]
            )

            # warm-up collective input (content irrelevant; 128B)
            wsrc = st.tile([1, 64], f16)
            nc.gpsimd.memset(wsrc[:], 0.0)
            wd = dram.tile([1, 64], f16, name="warm_d")
            nc.sync.dma_start(out=wd[:], in_=wsrc[:])
            wr = dram.tile([1, 64 * cfg.n_cores], f16, name="warm_r",
                           addr_space="Shared")
            rd = dram.tile([1, U], f16, name="ubf_all", addr_space="Shared")

            ubf_row = st.tile([1, W], f16)
            nc.gpsimd.memset(ubf_row[:], 0.0)
            ubf_bc = st.tile([128, W], f16)
            scores = st.tile([128, NCH], f32)

            # ---- main loop ----
            # Emission (= per-engine program) order matters: the first
            # `prefetch` gather-pairs go on Pool before the collectives (DMA
            # prefetches during the skew wait); all ubf_bc readers come after
            # partition_broadcast; Pool-side ubf-adds for the trailing chunks
            # are emitted after ALL gathers so a late ubf can never block a
            # gather (deadlock via buffer reuse).
            gks, aks = [], []

            def emit_gathers(k):
                gk = gpool.tile([128, 1, W], f16, name="gk")
                nc.gpsimd.dma_gather(
                    out_ap=gk[:], in_ap=sim_t[:],
                    idxs_ap=uidx_sb[:, k * ICN : (k + 1) * ICN],
                    num_idxs=CH, num_idxs_reg=CH, elem_size=W,
                    transpose=False, queue_num=0,
                )
                ak = apool.tile([128, 1, W], f16, name="ak")
                nc.gpsimd.dma_gather(
                    out_ap=ak[:], in_ap=rtt_t[:],
                    idxs_ap=iidx_sb[:, k * ICN : (k + 1) * ICN],
                    num_idxs=CH, num_idxs_reg=CH, elem_size=W,
                    transpose=False, queue_num=1,
                )
                gks.append(gk)
                aks.append(ak)

            for k in range(cfg.prefetch):
                emit_gathers(k)
            # warm-up cc absorbs core start skew, then the real exchange
            nc.gpsimd.collective_compute(
                "AllGather", Alu.bypass, replica_groups=groups,
                ins=[wd.opt()], outs=[wr.opt()],
            )
            nc.gpsimd.collective_compute(
                "AllGather", Alu.bypass, replica_groups=groups,
                ins=[pd.opt()], outs=[rd.opt()],
            )
            nc.sync.dma_start(out=ubf_row[:, :U], in_=rd[:])
            nc.gpsimd.partition_broadcast(out_ap=ubf_bc[:], in_ap=ubf_row[:])

            for k in range(NCH):
                if k + cfg.prefetch < NCH:
                    emit_gathers(k + cfg.prefetch)
                gk, ak = gks[k], aks[k]
                # products: ak += ubf_bc; ak *= gk (in-place); the add runs
                # on Pool for the trailing chunks to offload DVE
                eng = nc.gpsimd if k >= NCH - cfg.pool_adds else nc.vector
                eng.tensor_tensor(
                    out=ak[:, 0, :], in0=ak[:, 0, :], in1=ubf_bc[:],
                    op=Alu.add,
                )
                nc.vector.tensor_tensor(
                    out=ak[:, 0, :], in0=ak[:, 0, :], in1=gk[:, 0, :],
                    op=Alu.mult,
                )
                # the 8320-long row dot: ACT Copy + fp32 accumulator (the
                # copy itself lands in the dead gk tile)
                nc.scalar.activation(
                    out=gk[:, 0, :], in_=ak[:, 0, :], func=Act.Copy,
                    accum_out=scores[:, k : k + 1],
                )

            # ---- finish: 5*sigmoid, transpose, one contiguous output DMA
            fin = st.tile([128, NCH], f32)
            nc.scalar.activation(out=fin[:], in_=scores[:], func=Act.Sigmoid)
            nc.vector.tensor_scalar_mul(out=fin[:], in0=fin[:], scalar1=5.0)
            ps_o = psp.tile([NCH, 128], f32, name="ps_o")
            nc.tensor.transpose(out=ps_o[:], in_=fin[:], identity=ident[:])
            fin_t = st.tile([NCH, 128], f32)
            nc.vector.tensor_copy(out=fin_t[:], in_=ps_o[:])
            nc.sync.dma_start(
                out=out_t[:].rearrange("(k p) -> k p", p=128), in_=fin_t[:]
            )

    nc.compile()
    return nc


def make_in_maps(cfg, user, item, rating_mtx, user_similarity, user_bias, item_bias, global_bias):
    import ml_dtypes

    U, I, B, W, BL = cfg.n_users, cfg.n_items, cfg.batch, cfg.w, cfg.bl
    f16 = np.float16
    f8 = ml_dtypes.float8_e4m3
    u_i = np.asarray(user).astype(np.int64)
    i_i = np.asarray(item).astype(np.int64)
    sim = np.asarray(user_similarity, dtype=np.float32)
    R = np.asarray(rating_mtx, dtype=np.float32)
    ub = np.asarray(user_bias, dtype=np.float32)
    ibg = np.asarray(item_bias, dtype=np.float32) + np.float32(
        np.asarray(global_bias)
    )

    def hilo(x):
        hi = x.astype(f16)
        lo = (x - hi.astype(np.float32)).astype(f16)
        return hi, lo

    ub_hi, ub_lo = hilo(ub)
    ib_hi, ib_lo = hilo(ibg)

    # shared tables (identical on every core)
    sa = np.zeros((U, W), f16)
    sa[:, :U] = sim.astype(f16)
    sa[:, U] = ub_hi
    sa[:, U + 1] = ub_lo
    sa[:, U + 2] = 1.0
    sa[:, U + 3] = 1.0
    ra = np.zeros((I, W), f16)
    ra[:, :U] = (R.T - 2.5).astype(f16)
    ra[:, U] = 1.0
    ra[:, U + 1] = 1.0
    ra[:, U + 2] = ib_hi
    ra[:, U + 3] = ib_lo

    # idx layout: [16, n/16] block (idx i at [i%16, i//16]) tiled 8x down the
    # partition axis -- each GPSIMD Q7 core reads its own 16-partition replica
    def idx_table(v):
        return np.tile(v.astype(np.int16).reshape(-1, 16).T, (8, 1))

    rc = (R - 2.5).astype(f8)
    maps = []
    for k in range(cfg.n_cores):
        lo, hi = k * BL, (k + 1) * BL
        maps.append({
            "sim_aug": sa,
            "ratt_aug": ra,
            "rnat": rc[lo:hi],
            "uidx": idx_table(u_i[lo:hi]),
            "iidx": idx_table(i_i[lo:hi]),
        })
    return maps


_PROGRAM_CACHE = {}


def _get_program(cfg: Cfg):
    if cfg not in _PROGRAM_CACHE:
        _PROGRAM_CACHE[cfg] = build_program(cfg)
    return _PROGRAM_CACHE[cfg]


def assemble_out(cfg, results):
    return np.concatenate(
        [np.asarray(results[k]["out"], dtype=np.float32).reshape(cfg.bl)
         for k in range(cfg.n_cores)]
    )


def kernel(user, item, rating_mtx, user_similarity, user_bias, item_bias, global_bias):
    from concourse import bass_utils

    cfg = Cfg()
    assert np.asarray(rating_mtx).shape == (cfg.n_users, cfg.n_items)
    assert np.asarray(user).shape == (cfg.batch,)
    nc = _get_program(cfg)
    in_maps = make_in_maps(
        cfg, user, item, rating_mtx, user_similarity, user_bias, item_bias, global_bias
    )
    res = bass_utils.run_bass_kernel_spmd(
        nc, in_maps, core_ids=list(range(cfg.n_cores))
    )
    return assemble_out(cfg, res.results)


# revision 15
# speedup vs baseline: 1.3453x; 1.3453x over previous
"""Trainium2 Bass kernel: collaborative-filtering score (segment_reduce problem).

Math (per batch element b):
    ubf[u]    = masked mean over nonzero entries of rating_mtx[u, :]
    score[b]  = sum_u  S[user_b, u] * (R[u, item_b] - ubf[u])
    out[b]    = 5 * sigmoid(score[b] + user_bias[user_b] + item_bias[item_b] + gb)

Distribution: BATCH-sharded. Core k owns batch slice [k*1024, (k+1)*1024) and
gathers FULL 8192-wide fp16 rows of the (replicated) augmented similarity
table (by user idx) and of the transposed rating table (by item idx) with
NON-transposed dma_gather: each row lands whole on one partition, so a chunk
is [128 batch-rows x 8320 cols] and the score is a plain FREE-AXIS reduce:
    DVE/Pool:  ak += ubf_bc   (row-broadcast -(ubf-2.5), built once)
    DVE:       ak *= gk       (in-place products)
    ACT:       Copy+accum_out -> scores[:, k]  (the 8192-long dot, fp32 accum)
No PE matmuls, no score AllReduce: each core sigmoids its own scores and
writes its output shard (one transposed [8,128] contiguous DMA); the host
concatenates. 8x fewer gathered rows than user-sharding keeps the Pool
engine (~3us fixed per dma_gather) off the critical path.

ubf is computed u-sharded per the hint ("masked mean shards along n_users
rows with an all-gather of user_bias_fixed"): each core streams its fp8
natural-layout slice (R-2.5, exact in e4m3) split across the sync AND scalar
DMA rings (so the gather queues don't starve it), sums on ACT (Copy+accum),
counts nonzeros on DVE (mask+reduce) for most tiles and on ACT (Sign(x+2)
accum = 2cnt-I) for the rest, PE-transposes the [128,8] result to user
order, and a tiny fp16 AllGather distributes it. A dummy 256B AllGather is
issued first, AFTER the first `prefetch` chunk gather-pairs are queued on
Pool: the first collective absorbs the multi-core start skew while the DMA
queues prefetch ~17MB of gather data + the rnat stream, so the real
exchange costs only ~5us on an already-synced device.

Tables are fp16 (NOT bf16): S values are ~N(0, 0.011) so fp16's 10-bit
mantissa cuts quantization noise ~8x vs bf16; centered ratings and the
hi/lo-split biases are exact. Biases ride as 4 augmented fp16 columns
(8192..8195) folded into the same dot product.
"""

import sys
from dataclasses import dataclass

import numpy as np

if "/opt/trn_rl_repo" not in sys.path:
    sys.path.insert(0, "/opt/trn_rl_repo")


@dataclass(frozen=True)
class Cfg:
    n_users: int = 8192
    n_items: int = 4096
    batch: int = 8192
    n_cores: int = 8
    chunk: int = 128  # batch elems per gather chunk
    prefetch: int = 3  # gather chunk-pairs queued on Pool before the warm-up cc
    act_cnts: int = 4  # trailing rnat tiles whose count runs on ACT, not DVE

    @property
    def bl(self) -> int:  # batch elems per core
        return self.batch // self.n_cores

    @property
    def w(self) -> int:
        # gather-row width: n_users data cols + 4 bias cols, padded to a
        # multiple of 128 elems (fp16 -> 256B rows, dma_gather constraint)
        return ((self.n_users + 4 + 127) // 128) * 128


def build_program(cfg: Cfg):
    from concourse import bacc, mybir, tile
    from concourse.masks import make_identity

    f32 = mybir.dt.float32
    f16 = mybir.dt.float16
    f8 = mybir.dt.float8e4
    i16 = mybir.dt.int16
    Alu = mybir.AluOpType
    Act = mybir.ActivationFunctionType

    U, I, B, W = cfg.n_users, cfg.n_items, cfg.batch, cfg.w
    BL, CH = cfg.bl, cfg.chunk
    NCH = BL // CH  # main-loop chunks (8)
    ICN = CH // 16  # idx-table cols per chunk
    RT = BL // 128  # rnat row-tiles (8)
    IDXC = BL // 16
    groups = [list(range(cfg.n_cores))]

    nc = bacc.Bacc(
        None, target_bir_lowering=False, debug=False, num_swdge_queues=2
    )

    sim_t = nc.dram_tensor("sim_aug", [U, W], f16, kind="ExternalInput")
    rtt_t = nc.dram_tensor("ratt_aug", [I, W], f16, kind="ExternalInput")
    rnat_t = nc.dram_tensor("rnat", [BL, I], f8, kind="ExternalInput")
    uidx_t = nc.dram_tensor("uidx", [128, IDXC], i16, kind="ExternalInput")
    iidx_t = nc.dram_tensor("iidx", [128, IDXC], i16, kind="ExternalInput")
    out_t = nc.dram_tensor("out", [BL], f32, kind="ExternalOutput")

    with tile.TileContext(nc) as tc:
        with (
            tc.tile_pool(name="static", bufs=1) as st,
            tc.tile_pool(name="rstream", bufs=1) as rpool,
            tc.tile_pool(name="scr", bufs=1) as scr,
            tc.tile_pool(name="scr2", bufs=1) as scr2,
            tc.tile_pool(name="gpool", bufs=4) as gpool,
            tc.tile_pool(name="apool", bufs=3) as apool,
            tc.tile_pool(name="ps", bufs=1, space="PSUM") as psp,
            tc.tile_pool(name="dram", bufs=1, space="DRAM") as dram,
        ):
            # ---- static setup ----
            two_b = st.tile([128, 1], f32)
            nc.gpsimd.memset(two_b[:], 2.0)
            ident = st.tile([128, 128], f32)
            make_identity(nc, ident[:])
            uidx_sb = st.tile([128, IDXC], i16)
            nc.sync.dma_start(out=uidx_sb[:], in_=uidx_t[:])
            iidx_sb = st.tile([128, IDXC], i16)
            nc.sync.dma_start(out=iidx_sb[:], in_=iidx_t[:])

            # ---- rnat stream: all tiles up-front, alternating sync/scalar
            # DMA rings so the two gather queues can't starve it
            rts = []
            for j in range(RT):
                rt = rpool.tile([128, I], f8, name=f"rt{j}")
                eng = nc.sync if j % 2 == 0 else nc.scalar
                eng.dma_start(out=rt[:], in_=rnat_t[j * 128 : (j + 1) * 128, :])
                rts.append(rt)

            # ---- ubf local pass: sum on ACT; count on DVE (mask+reduce)
            # except the last `act_cnts` tiles, counted on ACT via
            # Sign(x+2.0): accum = 2*cnt - I (x is never 0).
            sum_acc = st.tile([128, RT], f32)
            cnt_acc = st.tile([128, RT], f32)
            dve_cnts = RT - cfg.act_cnts
            for j in range(RT):
                rt = rts[j]
                s1 = scr.tile([128, I], f8, name="s1")
                nc.scalar.activation(
                    out=s1[:], in_=rt[:], func=Act.Copy,
                    accum_out=sum_acc[:, j : j + 1],
                )
                if j < dve_cnts:
                    mk = scr2.tile([128, I], f16, name="mk")
                    nc.vector.tensor_scalar(
                        out=mk[:], in0=rt[:], scalar1=-2.5, scalar2=None,
                        op0=Alu.not_equal,
                    )
                    nc.vector.tensor_reduce(
                        out=cnt_acc[:, j : j + 1], in_=mk[:],
                        axis=mybir.AxisListType.X, op=Alu.add,
                    )
                else:
                    s2 = scr.tile([128, I], f8, name="s1")
                    nc.scalar.activation(
                        out=s2[:], in_=rt[:], func=Act.Sign, bias=two_b[:],
                        accum_out=cnt_acc[:, j : j + 1],
                    )
            if cfg.act_cnts:
                # convert the Sign-accumulated cols: cnt = (acc + I) / 2
                nc.vector.tensor_scalar(
                    out=cnt_acc[:, dve_cnts:], in0=cnt_acc[:, dve_cnts:],
                    scalar1=0.5, scalar2=float(I) / 2, op0=Alu.mult,
                    op1=Alu.add,
                )
            # ubf math (values are R-2.5; sum_R = sum_acc + 2.5*I):
            #   ubf  = sum_R / max(cnt, 1)      (0 when cnt==0)
            #   ubfn = -(ubf - 2.5) = 2.5 - ubf (cnt==0 -> sum_R==0 -> 2.5,
            #   matching the reference's adjusted = R - 0 in centered form)
            cntm = st.tile([128, RT], f32)
            nc.vector.tensor_scalar(
                out=cntm[:], in0=cnt_acc[:], scalar1=1.0, scalar2=None,
                op0=Alu.max,
            )
            nc.vector.reciprocal(out=cntm[:], in_=cntm[:])
            ubfn = st.tile([128, RT], f32)
            nc.vector.tensor_scalar(
                out=ubfn[:], in0=sum_acc[:], scalar1=2.5 * I, scalar2=None,
                op0=Alu.add,
            )
            nc.vector.tensor_tensor(
                out=ubfn[:], in0=ubfn[:], in1=cntm[:], op=Alu.mult
            )
            nc.vector.tensor_scalar(
                out=ubfn[:], in0=ubfn[:], scalar1=-1.0, scalar2=2.5,
                op0=Alu.mult, op1=Alu.add,
            )
            # transpose [128, RT] -> [RT, 128] so DRAM order is user order
            ps_t = psp.tile([RT, 128], f32, name="ps_t")
            nc.tensor.transpose(out=ps_t[:], in_=ubfn[:], identity=ident[:])
            ubfn_t = st.tile([RT, 128], f16)
            nc.vector.tensor_copy(out=ubfn_t[:], in_=ps_t[:])
            pd = dram.tile([1, BL], f16, name="ubf_part")
            nc.sync.dma_start(
                out=pd[:].rearrange("o (j p) -> (o j) p", j=RT), in_=ubfn_t[:]
            )

            # warm-up collective input (content irrelevant; 128B)
            wsrc = st.tile([1, 64], f16)
            nc.gpsimd.memset(wsrc[:], 0.0)
            wd = dram.tile([1, 64], f16, name="warm_d")
            nc.sync.dma_start(out=wd[:], in_=wsrc[:])
            wr = dram.tile([1, 64 * cfg.n_cores], f16, name="warm_r",
                           addr_space="Shared")
            rd = dram.tile([1, U], f16, name="ubf_all", addr_space="Shared")

            ubf_row = st.tile([1, W], f16)
            nc.gpsimd.memset(ubf_row[:], 0.0)
            ubf_bc = st.tile([128, W], f16)
            scores = st.tile([128, NCH], f32)

            # ---- main loop ----
            # Emission (= per-engine program) order matters: the first
            # `prefetch` gather-pairs go on Pool before the collectives (DMA
            # prefetches during the skew wait); all ubf_bc readers come after
            # partition_broadcast; Pool-side ubf-adds for the trailing chunks
            # are emitted after ALL gathers so a late ubf can never block a
            # gather (deadlock via buffer reuse).
            gks, aks = [], []

            def emit_gathers(k):
                gk = gpool.tile([128, 1, W], f16, name="gk")
                nc.gpsimd.dma_gather(
                    out_ap=gk[:], in_ap=sim_t[:],
                    idxs_ap=uidx_sb[:, k * ICN : (k + 1) * ICN],
                    num_idxs=CH, num_idxs_reg=CH, elem_size=W,
                    transpose=False, queue_num=0,
                )
                ak = apool.tile([128, 1, W], f16, name="ak")
                nc.gpsimd.dma_gather(
                    out_ap=ak[:], in_ap=rtt_t[:],
                    idxs_ap=iidx_sb[:, k * ICN : (k + 1) * ICN],
                    num_idxs=CH, num_idxs_reg=CH, elem_size=W,
                    transpose=False, queue_num=1,
                )
                gks.append(gk)
                aks.append(ak)

            for k in range(cfg.prefetch):
                emit_gathers(k)
            # warm-up cc absorbs core start skew, then the real exchange
            nc.gpsimd.collective_compute(
                "AllGather", Alu.bypass, replica_groups=groups,
                ins=[wd.opt()], outs=[wr.opt()],
            )
            nc.gpsimd.collective_compute(
                "AllGather", Alu.bypass, replica_groups=groups,
                ins=[pd.opt()], outs=[rd.opt()],
            )
            nc.sync.dma_start(out=ubf_row[:, :U], in_=rd[:])
            # broadcast in halves so the first half-adds can start ~6us sooner
            H = W // 2
            nc.gpsimd.partition_broadcast(
                out_ap=ubf_bc[:, :H], in_ap=ubf_row[:, :H]
            )
            nc.gpsimd.partition_broadcast(
                out_ap=ubf_bc[:, H:], in_ap=ubf_row[:, H:]
            )

            for k in range(NCH):
                if k + cfg.prefetch < NCH:
                    emit_gathers(k + cfg.prefetch)
                gk, ak = gks[k], aks[k]
                # products: ak += ubf_bc (half-split so the lo half only
                # waits on the first partition_broadcast); ak *= gk
                nc.vector.tensor_tensor(
                    out=ak[:, 0, :H], in0=ak[:, 0, :H], in1=ubf_bc[:, :H],
                    op=Alu.add,
                )
                nc.vector.tensor_tensor(
                    out=ak[:, 0, H:], in0=ak[:, 0, H:], in1=ubf_bc[:, H:],
                    op=Alu.add,
                )
                nc.vector.tensor_tensor(
                    out=ak[:, 0, :], in0=ak[:, 0, :], in1=gk[:, 0, :],
                    op=Alu.mult,
                )
                # the 8320-long row dot: ACT Copy + fp32 accumulator (the
                # copy itself lands in the dead gk tile)
                nc.scalar.activation(
                    out=gk[:, 0, :], in_=ak[:, 0, :], func=Act.Copy,
                    accum_out=scores[:, k : k + 1],
                )

            # ---- finish: 5*sigmoid, transpose, one contiguous output DMA
            fin = st.tile([128, NCH], f32)
            nc.scalar.activation(out=fin[:], in_=scores[:], func=Act.Sigmoid)
            nc.vector.tensor_scalar_mul(out=fin[:], in0=fin[:], scalar1=5.0)
            ps_o = psp.tile([NCH, 128], f32, name="ps_o")
            nc.tensor.transpose(out=ps_o[:], in_=fin[:], identity=ident[:])
            fin_t = st.tile([NCH, 128], f32)
            nc.vector.tensor_copy(out=fin_t[:], in_=ps_o[:])
            nc.sync.dma_start(
                out=out_t[:].rearrange("(k p) -> k p", p=128), in_=fin_t[:]
            )

    nc.compile()
    return nc


def make_in_maps(cfg, user, item, rating_mtx, user_similarity, user_bias, item_bias, global_bias):
    import ml_dtypes

    U, I, B, W, BL = cfg.n_users, cfg.n_items, cfg.batch, cfg.w, cfg.bl
    f16 = np.float16
    f8 = ml_dtypes.float8_e4m3
    u_i = np.asarray(user).astype(np.int64)
    i_i = np.asarray(item).astype(np.int64)
    sim = np.asarray(user_similarity, dtype=np.float32)
    R = np.asarray(rating_mtx, dtype=np.float32)
    ub = np.asarray(user_bias, dtype=np.float32)
    ibg = np.asarray(item_bias, dtype=np.float32) + np.float32(
        np.asarray(global_bias)
    )

    def hilo(x):
        hi = x.astype(f16)
        lo = (x - hi.astype(np.float32)).astype(f16)
        return hi, lo

    ub_hi, ub_lo = hilo(ub)
    ib_hi, ib_lo = hilo(ibg)

    # shared tables (identical on every core)
    sa = np.zeros((U, W), f16)
    sa[:, :U] = sim.astype(f16)
    sa[:, U] = ub_hi
    sa[:, U + 1] = ub_lo
    sa[:, U + 2] = 1.0
    sa[:, U + 3] = 1.0
    ra = np.zeros((I, W), f16)
    ra[:, :U] = (R.T - 2.5).astype(f16)
    ra[:, U] = 1.0
    ra[:, U + 1] = 1.0
    ra[:, U + 2] = ib_hi
    ra[:, U + 3] = ib_lo

    # idx layout: [16, n/16] block (idx i at [i%16, i//16]) tiled 8x down the
    # partition axis -- each GPSIMD Q7 core reads its own 16-partition replica
    def idx_table(v):
        return np.tile(v.astype(np.int16).reshape(-1, 16).T, (8, 1))

    rc = (R - 2.5).astype(f8)
    maps = []
    for k in range(cfg.n_cores):
        lo, hi = k * BL, (k + 1) * BL
        maps.append({
            "sim_aug": sa,
            "ratt_aug": ra,
            "rnat": rc[lo:hi],
            "uidx": idx_table(u_i[lo:hi]),
            "iidx": idx_table(i_i[lo:hi]),
        })
    return maps


_PROGRAM_CACHE = {}


def _get_program(cfg: Cfg):
    if cfg not in _PROGRAM_CACHE:
        _PROGRAM_CACHE[cfg] = build_program(cfg)
    return _PROGRAM_CACHE[cfg]


def assemble_out(cfg, results):
    return np.concatenate(
        [np.asarray(results[k]["out"], dtype=np.float32).reshape(cfg.bl)
         for k in range(cfg.n_cores)]
    )


def kernel(user, item, rating_mtx, user_similarity, user_bias, item_bias, global_bias):
    from concourse import bass_utils

    cfg = Cfg()
    assert np.asarray(rating_mtx).shape == (cfg.n_users, cfg.n_items)
    assert np.asarray(user).shape == (cfg.batch,)
    nc = _get_program(cfg)
    in_maps = make_in_maps(
        cfg, user, item, rating_mtx, user_similarity, user_bias, item_bias, global_bias
    )
    res = bass_utils.run_bass_kernel_spmd(
        nc, in_maps, core_ids=list(range(cfg.n_cores))
    )
    return assemble_out(cfg, res.results)
